# revision 1
# baseline (speedup 1.0000x reference)
"""GCN message-passing kernel for Trainium2 (8 NeuronCores).

Problem: x [4,4096,64] f32, graph [4,4096,4096] f32, W [64,256], b [64].
  g = graph + I;  d = 1/(sqrt(g.sum(-1)) + 1e-7);  A = D g D
  h_{k+1} = A h_k (3 layers);  out = concat([x,h1,h2,h3], -1) @ W.T + b

Strategy (all sizes hardcoded):
  - 4 groups of 2 cores; group g handles batch element g; each core owns
    2048 graph rows (its output nodes).
  - Host passes each core its graph shard PRE-TRANSPOSED (j-major) so the
    contraction index lands on SBUF partitions; the shard is streamed from
    HBM once, cast to fp16, and kept resident in SBUF (16MB) for all 3
    propagation layers.
  - Row sums (for the normalization d) are computed during the load via
    ones-vector matmuls on the otherwise idle TensorEngine.
  - Normalization is folded into per-node vector scalings (u = d*h); the
    identity term is h += u (local rows).  Per layer the 2 cores of a group
    exchange their half of u via an AllGather collective.
  - Final linear runs in fp32 off the transposed concat features.
"""

import os
import sys

for _p in ("/opt/trn_rl_repo", "/opt/pypackages"):
    if _p not in sys.path:
        sys.path.insert(0, _p)

import numpy as np

import concourse.bass as bass
import concourse.mybir as mybir
from concourse import tile
from concourse.bass_utils import run_bass_kernel_spmd

F32 = mybir.dt.float32
F16 = mybir.dt.float16

B = 4          # batch
N = 4096       # nodes
D = 64         # feature dim
DEPTH = 3
NCORES = 8
ROWS = N // 2          # rows (output nodes) per core
RT = ROWS // 128       # 16 row tiles per core
JT = N // 128          # 32 contraction tiles
IB = ROWS // 512       # 4 i-blocks of 512 for matmul free dim

_MAX_DRAIN_WAITS = 1   # this walrus build encodes at most 1 sem-wait per CTRL inst


def _split_drain_waits(nc):
    """This walrus build encodes at most one sem-wait per instruction for
    several instruction structs; hoist excess waits onto injected
    same-engine Drain instructions placed immediately before."""
    n_split = 0
    for bb in nc.main_func.blocks:
        il = bb.instructions  # live list
        i = 0
        while i < len(il):
            ins = il[i]
            si = getattr(ins, "sync_info", None)
            if (si is not None and getattr(ins, "engine", None) is not None
                    and len(si.on_wait) > _MAX_DRAIN_WAITS):
                n_split += 1
                waits = list(si.on_wait)
                pre = []
                k = 0
                while len(waits) - k > _MAX_DRAIN_WAITS:
                    chunk = waits[k:k + _MAX_DRAIN_WAITS]
                    k += _MAX_DRAIN_WAITS
                    pre.append(mybir.InstDrain(
                        name=f"{ins.name}-sw{len(pre)}",
                        opcode="Drain",
                        engine=ins.engine,
                        debug=ins.debug,
                        ins=[], outs=[],
                        sync_info=mybir.SyncInfo(on_wait=chunk, on_update=[]),
                    ))
                ins.sync_info = mybir.SyncInfo(
                    on_wait=waits[k:], on_update=list(si.on_update))
                for j, d in enumerate(pre):
                    il.insert(i + j, d)
                i += len(pre)
            i += 1


def _build_program():
    nc = bass.Bass(trn_type="TRN2", num_devices=NCORES)

    tg = nc.dram_tensor("tg", [N, ROWS], F32, kind="ExternalInput")       # graph[b].T columns (own rows)
    xt = nc.dram_tensor("xt", [D, ROWS], F32, kind="ExternalInput")       # x[b].T own columns
    xf = nc.dram_tensor("xf", [N, D], F32, kind="ExternalInput")          # x[b] full, natural layout
    wt = nc.dram_tensor("wt", [2, 128, D], F32, kind="ExternalInput")     # W.T as two [128,64] K-tiles
    bvec = nc.dram_tensor("bvec", [1, D], F32, kind="ExternalInput")
    ident = nc.dram_tensor("ident", [128, 128], F16, kind="ExternalInput")
    out = nc.dram_tensor("out", [ROWS, D], F32, kind="ExternalOutput")

    groups = [[2 * g, 2 * g + 1] for g in range(B)]

    with tile.TileContext(nc) as tc:
        with tc.tile_pool(name="res", bufs=1) as res_pool, \
             tc.tile_pool(name="stage", bufs=3) as stage_pool, \
             tc.tile_pool(name="small", bufs=1) as small_pool, \
             tc.tile_pool(name="uf16", bufs=2) as u_pool, \
             tc.tile_pool(name="psacc", bufs=6, space="PSUM") as psacc, \
             tc.tile_pool(name="pssm", bufs=2, space="PSUM") as pssm, \
             tc.tile_pool(name="outp", bufs=2) as out_pool, \
             tc.tile_pool(name="dram", bufs=1, space="DRAM") as dram_pool:

            # ---- small constants ----
            id_f16 = small_pool.tile([128, 128], F16, tag="idf16")
            nc.sync.dma_start(id_f16[:], ident[:])
            wt_sb = small_pool.tile([128, 2 * D], F32, tag="wt")
            nc.sync.dma_start(wt_sb[:, 0:D], wt[0])
            nc.sync.dma_start(wt_sb[:, D:2 * D], wt[1])
            b_sb = small_pool.tile([1, D], F32, tag="bsb")
            nc.sync.dma_start(b_sb[:], bvec[:])
            ones_row_f32 = small_pool.tile([1, 128], F32, tag="ones32")
            nc.vector.memset(ones_row_f32[:], 1.0)
            ones_col_f16 = small_pool.tile([128, 1], F16, tag="ones16")
            nc.vector.memset(ones_col_f16[:], 1.0)

            # b replicated across partitions: ones[1,128].T @ b[1,64]
            ps_b = pssm.tile([128, D], F32, tag="sm")
            nc.tensor.matmul(ps_b[:], ones_row_f32[:], b_sb[:])
            b_rep = small_pool.tile([128, D], F32, tag="brep")
            nc.scalar.copy(b_rep[:], ps_b[:])

            # cat^T feature rows: cat1 = [x^T; h1^T], cat2 = [h2^T; h3^T]
            cat1 = small_pool.tile([128, ROWS], F32, tag="cat1")
            cat2 = small_pool.tile([128, ROWS], F32, tag="cat2")
            nc.sync.dma_start(cat1[0:D, :], xt[:])
            # full x in natural layout: tile jt at [:, jt*D:(jt+1)*D]
            x_full = small_pool.tile([128, JT * D], F32, tag="xfull")
            nc.sync.dma_start(x_full[:].rearrange("p (t d) -> p t d", d=D),
                              xf[:].rearrange("(t p) d -> p t d", p=128))

            # ---- load graph^T shard: stream fp32, cast to resident fp16,
            #      accumulate row sums on the TensorEngine ----
            resident = res_pool.tile([128, JT * ROWS], F16, tag="resident")
            ps_rs = [psacc.tile([1, 512], F32, tag="acc", name=f"rs{ib}")
                     for ib in range(IB)]
            for jt in range(JT):
                st = stage_pool.tile([128, ROWS], F32, tag="stage")
                nc.sync.dma_start(st[:], tg[jt * 128:(jt + 1) * 128, :])
                rslice = resident[:, jt * ROWS:(jt + 1) * ROWS]
                eng = nc.vector if jt % 2 == 0 else nc.scalar
                if eng is nc.vector:
                    eng.tensor_copy(rslice, st[:])
                else:
                    eng.copy(rslice, st[:])
                for ib in range(IB):
                    nc.tensor.matmul(
                        ps_rs[ib][:],
                        ones_col_f16[:],
                        rslice[:, ib * 512:(ib + 1) * 512],
                        start=(jt == 0), stop=(jt == JT - 1),
                    )

            # ---- normalization: d = 1/(sqrt(rowsum + 1) + 1e-7) ----
            # replicate raw rowsums over all 128 partitions first (PE outer
            # product), then compute d = 1/(sqrt(s+1)+eps) at full width.
            scr1 = stage_pool.tile([128, ROWS], F32, tag="stage", name="dscr1")
            d_rep = small_pool.tile([128, ROWS], F32, tag="drep")
            for ib in range(IB):
                sl = slice(ib * 512, (ib + 1) * 512)
                s_row = scr1[0:1, sl]
                nc.scalar.copy(s_row, ps_rs[ib][:])
                ps_d = pssm.tile([128, 512], F32, tag="sm", name=f"psd{ib}")
                nc.tensor.matmul(ps_d[:], ones_row_f32[:], s_row)
                # sqrt(s + 1) with bias, then +eps, then reciprocal — all at
                # full 128-partition width, ping-ponging to avoid in-place
                nc.scalar.activation(d_rep[:, sl], ps_d[:],
                                     mybir.ActivationFunctionType.Sqrt, bias=1.0)
                nc.vector.tensor_scalar_add(scr1[:, sl], d_rep[:, sl], 1e-7)
                nc.vector.reciprocal(d_rep[:, sl], scr1[:, sl])

            # ---- u0 = d * x (transposed layout, fp16) ----
            u_own_T = u_pool.tile([D, ROWS], F16, tag="uT")
            nc.vector.tensor_tensor(u_own_T[:], cat1[0:D, :], d_rep[0:D, :],
                                    mybir.AluOpType.mult)

            # u exchange is chunked: each chunk c covers ROWS/CH own nodes
            # (row tiles c*TPC..) and AllGathers to j-tiles {c*TPC..} and
            # {JT/2 + c*TPC..} of u_full, so the next layer's K-accumulation
            # can start as soon as chunk 0 lands.
            CH = 2
            TPC = RT // CH  # row tiles per chunk

            def exchange_chunk(u_T_f16, u_full, xtag, c):
                u_nat = out_pool.tile([128, TPC * D], F16, tag="unat",
                                      name=f"unat{xtag}_{c}")
                for k in range(TPC):
                    it = c * TPC + k
                    ps_tr = pssm.tile([128, D], F16, tag="sm",
                                      name=f"pstr{xtag}_{it}")
                    nc.tensor.transpose(
                        ps_tr[:], u_T_f16[:, it * 128:(it + 1) * 128],
                        id_f16[0:D, 0:D])
                    nc.scalar.copy(u_nat[:, k * D:(k + 1) * D], ps_tr[:])
                snd = dram_pool.tile([TPC * 128, D], F16,
                                     name=f"snd{xtag}_{c}", tag=f"snd{xtag}_{c}")
                rcv = dram_pool.tile([2 * TPC * 128, D], F16,
                                     name=f"rcv{xtag}_{c}", tag=f"rcv{xtag}_{c}")
                nc.gpsimd.dma_start(
                    snd[:].rearrange("(t p) d -> p t d", p=128),
                    u_nat[:].rearrange("p (t d) -> p t d", d=D))
                nc.gpsimd.collective_compute(
                    "AllGather", mybir.AluOpType.bypass,
                    replica_groups=groups,
                    ins=[snd[:].opt()], outs=[rcv[:].opt()])
                lo = slice((c * TPC) * D, (c * TPC + TPC) * D)
                hi = slice((JT // 2 + c * TPC) * D, (JT // 2 + c * TPC + TPC) * D)
                nc.gpsimd.dma_start(
                    u_full[:, lo].rearrange("p (t d) -> p t d", d=D),
                    rcv[0:TPC * 128, :].rearrange("(t p) d -> p t d", p=128))
                nc.gpsimd.dma_start(
                    u_full[:, hi].rearrange("p (t d) -> p t d", d=D),
                    rcv[TPC * 128:2 * TPC * 128, :].rearrange("(t p) d -> p t d", p=128))

            # j-tile order matching chunk arrival: chunk c delivers tiles
            # {c*TPC..c*TPC+TPC-1} (rank 0) and {JT/2+c*TPC..} (rank 1)
            jt_order = [jt for c in range(CH)
                        for jt in (list(range(c * TPC, (c + 1) * TPC))
                                   + list(range(JT // 2 + c * TPC,
                                                JT // 2 + (c + 1) * TPC)))]

            # u0 exchange shortcut: AllGather only the 8KB d vector, then
            # compute u0_full = d_full * x_full locally in natural layout
            # (per-partition scalar multiply, no transposes).
            u_full = u_pool.tile([128, JT * D], F16, tag="ufull", name="ufull0")
            d_snd = dram_pool.tile([ROWS, 1], F32, name="dsnd", tag="dsnd")
            d_rcv = dram_pool.tile([N, 1], F32, name="drcv", tag="drcv")
            nc.gpsimd.dma_start(d_snd[:].rearrange("(o r) v -> o (r v)", o=1),
                                d_rep[0:1, :])
            nc.gpsimd.collective_compute(
                "AllGather", mybir.AluOpType.bypass,
                replica_groups=groups,
                ins=[d_snd[:].opt()], outs=[d_rcv[:].opt()])
            d_full = small_pool.tile([128, JT], F32, tag="dfull")
            nc.gpsimd.dma_start(d_full[:],
                                d_rcv[:].rearrange("(t p) v -> p (t v)", p=128))
            for jt in range(JT):
                nc.vector.tensor_scalar_mul(
                    u_full[:, jt * D:(jt + 1) * D],
                    x_full[:, jt * D:(jt + 1) * D],
                    d_full[:, jt:jt + 1])

            # ---- propagation layers ----
            for layer in range(DEPTH):
                cat_dst = (cat1 if layer == 0 else cat2)
                roff = D if layer == 0 else (0 if layer == 1 else D)
                u_new_T = u_pool.tile([D, ROWS], F16, tag="uT",
                                      name=f"u_new_T{layer}")
                u_full_next = (None if layer == DEPTH - 1 else
                               u_pool.tile([128, JT * D], F16, tag="ufull",
                                           name=f"ufull{layer + 1}"))
                for ib in range(IB):
                    ps_h_ib = psacc.tile([D, 512], F32, tag="acc",
                                         name=f"psh{layer}_{ib}")
                    for idx, jt in enumerate(jt_order):
                        nc.tensor.matmul(
                            ps_h_ib[:],
                            u_full[:, jt * D:(jt + 1) * D],
                            resident[:, jt * ROWS + ib * 512: jt * ROWS + (ib + 1) * 512],
                            start=(idx == 0), stop=(idx == JT - 1),
                        )
                    # h = d*(mm + u_prev_own); cat row block; u_new = d*h
                    sl = slice(ib * 512, (ib + 1) * 512)
                    hslice = cat_dst[roff:roff + D, sl]
                    d_sl = d_rep[roff:roff + D, sl]
                    nc.vector.tensor_tensor(hslice, ps_h_ib[:], u_own_T[:, sl],
                                            mybir.AluOpType.add)
                    nc.vector.tensor_tensor(hslice, hslice, d_sl,
                                            mybir.AluOpType.mult)
                    nc.vector.tensor_tensor(u_new_T[:, sl], hslice, d_sl,
                                            mybir.AluOpType.mult)
                    if layer < DEPTH - 1 and ib % (IB // CH) == IB // CH - 1:
                        exchange_chunk(u_new_T, u_full_next, layer + 1,
                                       ib // (IB // CH))
                u_own_T = u_new_T
                u_full = u_full_next

            # ---- final linear: out = cat @ W.T + b ----
            for it in range(RT):
                ps_o = pssm.tile([128, D], F32, tag="sm", name=f"pso{it}")
                isl = slice(it * 128, (it + 1) * 128)
                nc.tensor.matmul(ps_o[:], cat1[:, isl], wt_sb[:, 0:D],
                                 start=True, stop=False)
                nc.tensor.matmul(ps_o[:], cat2[:, isl], wt_sb[:, D:2 * D],
                                 start=False, stop=True)
                o_sb = out_pool.tile([128, D], F32, tag="osb")
                nc.vector.tensor_tensor(o_sb[:], ps_o[:], b_rep[:],
                                        mybir.AluOpType.add)
                nc.sync.dma_start(out[isl, :], o_sb[:])

    _split_drain_waits(nc)
    return nc


_NC_CACHE = None


def _get_program():
    global _NC_CACHE
    if _NC_CACHE is None:
        _NC_CACHE = _build_program()
    return _NC_CACHE


def _prep_inputs(x, graph, W, b):
    wt_h = np.ascontiguousarray(W.T.reshape(2, 128, D).astype(np.float32))
    b_h = np.ascontiguousarray(b.reshape(1, D).astype(np.float32))
    ident = np.eye(128, dtype=np.float16)
    in_maps = []
    for c in range(NCORES):
        g, r = divmod(c, 2)
        rows = slice(r * ROWS, (r + 1) * ROWS)
        tg_c = np.ascontiguousarray(graph[g].T[:, rows])
        xt_c = np.ascontiguousarray(x[g].T[:, rows])
        xf_c = np.ascontiguousarray(x[g])
        in_maps.append({"tg": tg_c, "xt": xt_c, "xf": xf_c, "wt": wt_h,
                        "bvec": b_h, "ident": ident})
    return in_maps


def kernel(x, graph, W, b, trace=False, **kw):
    nc = _get_program()
    in_maps = _prep_inputs(np.asarray(x, np.float32), np.asarray(graph, np.float32),
                           np.asarray(W, np.float32), np.asarray(b, np.float32))
    res = run_bass_kernel_spmd(nc, in_maps, core_ids=list(range(NCORES)),
                               trace=trace, **kw)
    out = np.empty((B, N, D), np.float32)
    for c in range(NCORES):
        g, r = divmod(c, 2)
        out[g, r * ROWS:(r + 1) * ROWS, :] = res.results[c]["out"]
    if trace:
        kernel.last_exec_time_ns = res.exec_time_ns
        kernel.last_results = res
    return out

